# revision 35
# baseline (speedup 1.0000x reference)
# RWKV token-shift + LoRA mixing block for Trainium2, 8-core SPMD.
#
# Reference computation (per batch b, token t):
#   sx[t]     = x[t-1] - x[t]            (x[-1] comes from state row i1)
#   xk        = x + sx * time_maa_x
#   h         = tanh(xk @ w1)            # [T, 160]
#   xxx[f]    = h[:, 32f:32f+32] @ w2[f] # [T, D] for f in 0..4
#   out[t,f]  = x[t] + sx[t] * (maa_f + xxx[t,f])
#   new_state = state with row i1 := x[:, -1]
#
# Sharding: 8 cores = (batch b = c//2) x (sequence half = c%2), 1024 tokens
# per core.  Each 128-token tile's halo (token t0-1) comes from row 127 of
# the previous x tile via a one-hot PE matmul; tile 0 uses a DRAM halo row.
#
# Device strategy per core (T=1024, D=2048):
#  - sx via PE:  psum = (S - I) @ x_tile + halo row (e0/e127 matmuls, all
#    float32r = full-rate fp32 bits), ACT copy to SBUF.  No cross-partition
#    shifts ever touch the vector engines.
#  - xT (bf16) is built by 16 single-shot DMA xbar transposes from a
#    host-cast bf16 copy of x; a padded halo column keeps x[t-1] reads as a
#    free-dim offset (token columns stay 32B-aligned).
#  - stage 1 (hT = tanh(xk @ w1)) uses the identity
#        xk @ w1 = x @ (w1 - tmx*w1) + x_prev @ (tmx*w1)
#    with both weight halves host-packed bf16: no elementwise prep at all.
#    Tiny selection matmuls route each f's 32 pre-tanh rows into a [33, N]
#    block whose 33rd row becomes exactly 1.0 via the tanh per-partition
#    bias (tanh(20) == 1), so stage 2 adds maa_f through the packed
#    [w2_f; maa_f] 33rd row in a single K=33 matmul.
#  - final: DVE  out = psum * sx  (only DVE can read PSUM for TT); the
#    "+ x" add is split DVE / GPSIMD / PE(identity-matmul accumulate with
#    ACT copy-out) to spread elementwise work across every engine, then
#    HWDGE-stores go straight into out[t, f, :] (stores on the ACT ring,
#    loads + transposes on the sync ring).
import sys

if "/opt/trn_rl_repo" not in sys.path:
    sys.path.insert(0, "/opt/trn_rl_repo")

import numpy as np
import ml_dtypes

import concourse.bass as bass
import concourse.mybir as mybir
from concourse import bacc, tile
from concourse.bass_utils import run_bass_kernel_spmd

F32 = mybir.dt.float32
F32R = mybir.dt.float32r  # fp32 bits, reduced-precision multiply, 4x faster PE
BF16 = mybir.dt.bfloat16
TANH = mybir.ActivationFunctionType.Tanh

D = 2048
J = 160
NK = D // 128  # 16 d-chunks
HEAD = 64
N_CORES = 8

LAST_RESULT = None  # BassKernelResults of the most recent run (for profiling)


def build_tile_program(tc, io, T, tps=2):
    """Emit the per-core tile program.

    io: dict name -> bass.AP for dram tensors.
    T: tokens handled by this core.  tps: 128-token tiles per super-tile.
    """
    from contextlib import ExitStack
    ctx = ExitStack()
    nc = tc.nc
    nt = T // 128
    assert nt % tps == 0
    N = tps * 128  # stage-1/2 token block width

    x_d, out_d = io["x"], io["out"]

    wts = ctx.enter_context(tc.tile_pool(name="wts", bufs=1))
    xp = ctx.enter_context(tc.tile_pool(name="xp", bufs=min(nt, 6)))
    sxp = ctx.enter_context(tc.tile_pool(name="sxp", bufs=min(nt, tps + 2)))
    xtp = ctx.enter_context(tc.tile_pool(name="xtp", bufs=NK))
    htp = ctx.enter_context(tc.tile_pool(name="htp", bufs=2))
    outp = ctx.enter_context(tc.tile_pool(name="outp", bufs=5))
    pss = ctx.enter_context(
        tc.tile_pool(name="pss", bufs=4, space=bass.MemorySpace.PSUM))
    pso = ctx.enter_context(
        tc.tile_pool(name="pso", bufs=2, space=bass.MemorySpace.PSUM))

    def load_const(name, shape, dtype):
        t = wts.tile(shape, dtype, tag=name, name=name + "_sb")
        nc.sync.dma_start(t[:], io[name][:].bitcast(dtype))
        return t

    wpack = load_const("wpack", [128, 2 * NK * J], BF16)
    selp = load_const("selp", [128, 4 * 33], F32R)
    sel4 = load_const("sel4", [32, 33], F32R)
    w2all = load_const("w2all", [33, 5 * D], BF16)
    shiftm = load_const("shiftm", [128, 128], F32R)
    e0 = load_const("e0", [1, 128], F32R)
    e127 = load_const("e127", [128, 128], F32R)
    halo0 = load_const("halo0", [1, D], F32R)
    biasv = load_const("biasv", [128, 1], F32)
    ident = load_const("ident", [128, 128], F32R)
    haloT = load_const("haloT", [128, NK], BF16)

    # Resident transposed x, one [128, 16+T] bf16 tile per 128-wide d-chunk.
    # Column 16+t is token t; column 15 is the halo token (t = -1); cols
    # 0-14 pad so token columns stay 32-byte aligned for the xbar DMA.
    # Each tile is filled by ONE DMA-transpose from the host-cast bf16 x.
    xT = [xtp.tile([128, 16 + T], BF16, tag="xT", name=f"xT{k}")
          for k in range(NK)]
    xbf_d = io["xbf"]
    for k in range(NK):
        nc.sync.dma_start(xT[k][:, 16:16 + T],
                          xbf_d[:, k * 128:(k + 1) * 128], transpose=True)
        nc.scalar.copy(xT[k][:, 15:16], haloT[:, k:k + 1])

    x_tiles = [None] * nt
    sx_tiles = [None] * nt

    for i in range(nt):
        xi = xp.tile([128, D], F32R, tag="x")
        nc.sync.dma_start(xi[:], x_d[i * 128:(i + 1) * 128, :].bitcast(F32R))
        x_tiles[i] = xi

        # sx = (S - I) @ x + halo into row 0   (PE), then ACT copy to SBUF.
        # Tile 0's halo is the DRAM halo row (rank-1 via e0); later tiles
        # take row 127 of the previous x tile (one-hot e127 matmul).
        sxi = sxp.tile([128, D], F32, tag="sx")
        for c in range(4):
            ps = pss.tile([128, 512], F32, tag="ps")
            cs = slice(c * 512, (c + 1) * 512)
            nc.tensor.matmul(ps[:], shiftm[:], xi[:, cs],
                             start=True, stop=False)
            if i == 0:
                nc.tensor.matmul(ps[:], e0[:], halo0[0:1, cs],
                                 start=False, stop=True)
            else:
                nc.tensor.matmul(ps[:], e127[:], x_tiles[i - 1][:, cs],
                                 start=False, stop=True)
            nc.scalar.copy(sxi[:, cs], ps[:])
        sx_tiles[i] = sxi

        if i % tps != tps - 1:
            continue

        # ---- super-tile s complete: stage 1 then stage 2 ----
        # Each f's 32 hidden units land at partition base 0 or 64 of one of
        # three ht tensors, followed by a memset ones-row at base+32 so
        # stage 2 is a single K=33 matmul that adds maa_f via the packed
        # w2 ([w2_f; maa_f]) 33rd row.  f0||f1 (f2||f3) run in concurrent
        # PE row/col groups.
        s = i // tps

        psH = pss.tile([128, N], F32, tag="ps")
        psH2 = pss.tile([32, N], F32, tag="ps")
        for (mo, msz, pt_) in ((0, 128, psH), (128, 32, psH2)):
            nmm = 0
            for pass_ in range(2):      # 0: w1b against x,  1: w1p against x_prev
                off = 16 - pass_        # halo-padded column offset
                for k in range(NK):
                    c0 = (pass_ * NK + k) * J + mo
                    nc.tensor.matmul(
                        pt_[:],
                        wpack[:, c0:c0 + msz],
                        xT[k][:, s * N + off: s * N + off + N],
                        start=(nmm == 0), stop=(nmm == 2 * NK - 1))
                    nmm += 1
        hpre = htp.tile([128, N], F32R, tag="hpre")
        hpre2 = htp.tile([32, N], F32R, tag="hpre2")
        nc.scalar.copy(hpre[:], psH[:])
        nc.scalar.copy(hpre2[:], psH2[:])

        # Route each f's 32 rows into a base-0 [33, N] block via a tiny
        # selection matmul, then tanh (+bias -> ones row at row 32).
        hts = []
        for f in range(5):
            ps_t = pss.tile([33, N], F32, tag="ps", name=f"psel{f}")
            ht_t = htp.tile([33, N], BF16, tag="ht", name=f"ht{f}",
                            bufs=10)
            if f < 4:
                nc.tensor.matmul(ps_t[:], selp[:, 33 * f:33 * f + 33],
                                 hpre[:], start=True, stop=True)
            else:
                nc.tensor.matmul(ps_t[:], sel4[:], hpre2[:],
                                 start=True, stop=True)
            nc.scalar.activation(ht_t[:], ps_t[:], TANH,
                                 bias=biasv[0:33, 0:1])
            hts.append(ht_t)

        for ii in range(s * tps, (s + 1) * tps):
            tloc = (ii % tps) * 128
            outs = [outp.tile([128, D], F32, tag="out", name=f"out{ii}_{f}")
                    for f in range(5)]
            # Final combine route per (ii, f): the "+ x" add runs on DVE,
            # GPSIMD, or the PE (identity-matmul accumulate onto the
            # DVE-scaled psum, then ACT copies PSUM->SBUF) to spread the
            # elementwise work across every engine.
            route = {f: (ii * 5 + f) % 8 for f in range(5)}
            for h in range(2):
                hs = slice(h * 1024, (h + 1) * 1024)
                for pair in ((0, 1), (2, 3), (4,)):
                    pos = {f: pso.tile([128, 1024], F32, tag="po", name=f"po{f}")
                           for f in pair}
                    for f in pair:
                        for c in (2 * h, 2 * h + 1):
                            sl = slice((c % 2) * 512, (c % 2) * 512 + 512)
                            cs = slice(f * D + c * 512, f * D + (c + 1) * 512)
                            nc.tensor.matmul(
                                pos[f][:, sl],
                                hts[f][:, tloc:tloc + 128],
                                w2all[:, cs],
                                start=True, stop=True)
                    for f in pair:
                        if route[f] < 6:
                            nc.vector.tensor_mul(outs[f][:, hs], pos[f][:],
                                                 sx_tiles[ii][:, hs])
                        else:
                            # scale in place, accumulate x on PE, copy out
                            nc.vector.tensor_mul(pos[f][:], pos[f][:],
                                                 sx_tiles[ii][:, hs])
                            for c in (2 * h, 2 * h + 1):
                                sl = slice((c % 2) * 512, (c % 2) * 512 + 512)
                                nc.tensor.matmul(
                                    pos[f][:, sl], ident[:],
                                    x_tiles[ii][:, c * 512:(c + 1) * 512],
                                    start=False, stop=True,
                                    skip_group_check=True)
                            nc.scalar.copy(outs[f][:, hs], pos[f][:])
            for f in range(5):
                if route[f] < 6:
                    eng = nc.vector if route[f] < 2 else nc.gpsimd
                    eng.tensor_add(outs[f][:], outs[f][:],
                                   x_tiles[ii][:].bitcast(F32))
                # Stores ride the ACT HWDGE ring; loads/transposes use the
                # sync ring — two rings double the DMA issue bandwidth.
                nc.scalar.dma_start(out_d[ii * 128:(ii + 1) * 128, f, :],
                                    outs[f][:])
    ctx.close()


def host_pack_weights(time_maa_x, time_maa_w1, time_maa_w2,
                      maa_k, maa_w, maa_v, maa_r, maa_g):
    tmx = np.asarray(time_maa_x, np.float32)
    w1 = np.asarray(time_maa_w1, np.float32)
    w2 = np.asarray(time_maa_w2, np.float32)
    w1p = w1 * tmx[:, None]
    w1b = w1 - w1p
    # wpack[p, pass, k, j] = w1x[pass][k*128+p, j]
    wpack = (np.stack([w1b, w1p], 0)
             .reshape(2, NK, 128, J)
             .transpose(2, 0, 1, 3)
             .reshape(128, 2 * NK * J)
             .astype(ml_dtypes.bfloat16))
    # Selection matrices: route pre-tanh h rows 32f..32f+31 into a 33-row
    # block (33rd col zero; the tanh bias writes the ones row).
    selp = np.zeros((128, 4 * 33), np.float32)
    for f in range(4):
        for m in range(32):
            selp[32 * f + m, 33 * f + m] = 1.0
    sel4 = np.zeros((32, 33), np.float32)
    sel4[:32, :32] = np.eye(32)
    maas = np.stack([maa_k, maa_w, maa_v, maa_r, maa_g]).astype(np.float32)
    e127 = np.zeros((128, 128), np.float32)
    e127[127, 0] = 1.0
    biasv = np.zeros((128, 1), np.float32)
    biasv[[32, 96]] = 20.0  # tanh(20) == 1.0 -> ones row for K=33 matmuls

    w2all = np.zeros((33, 5 * D), np.float32)
    for f in range(5):
        w2all[0:32, f * D:(f + 1) * D] = w2[f]
        w2all[32, f * D:(f + 1) * D] = maas[f]

    consts = {
        "wpack": wpack,
        "w2all": w2all.astype(ml_dtypes.bfloat16),
        "shiftm": (np.eye(128, 128, 1) - np.eye(128)).astype(np.float32),
        "e0": np.eye(1, 128, dtype=np.float32),
        "e127": e127,
        "biasv": biasv,
        "ident": np.eye(128, dtype=np.float32),
        "selp": selp,
        "sel4": sel4,
    }
    return consts


def build_nc(T, tps=2):
    # Bacc (not bare Bass): its compile() runs move_matmul_waits_to_ldweights
    # and generate_event_semaphores, which split multi-semaphore waits to
    # satisfy the 1-wait-per-instruction TRN2 constraint walrus enforces.
    nc = bacc.Bacc("TRN2", target_bir_lowering=False, debug=False)
    nt = T // 128
    io = {
        "x": nc.dram_tensor("x", [T, D], F32, kind="ExternalInput").ap(),
        "xbf": nc.dram_tensor("xbf", [T, D], BF16, kind="ExternalInput").ap(),
        "halo0": nc.dram_tensor("halo0", [1, D], F32,
                                kind="ExternalInput").ap(),
        "haloT": nc.dram_tensor("haloT", [128, NK], BF16,
                                kind="ExternalInput").ap(),
        "wpack": nc.dram_tensor("wpack", [128, 2 * NK * J], BF16,
                                kind="ExternalInput").ap(),
        "selp": nc.dram_tensor("selp", [128, 4 * 33], F32,
                               kind="ExternalInput").ap(),
        "sel4": nc.dram_tensor("sel4", [32, 33], F32,
                               kind="ExternalInput").ap(),
        "w2all": nc.dram_tensor("w2all", [33, 5 * D], BF16,
                                kind="ExternalInput").ap(),
        "shiftm": nc.dram_tensor("shiftm", [128, 128], F32,
                                 kind="ExternalInput").ap(),
        "e0": nc.dram_tensor("e0", [1, 128], F32, kind="ExternalInput").ap(),
        "e127": nc.dram_tensor("e127", [128, 128], F32,
                               kind="ExternalInput").ap(),
        "biasv": nc.dram_tensor("biasv", [128, 1], F32,
                                kind="ExternalInput").ap(),
        "ident": nc.dram_tensor("ident", [128, 128], F32,
                                kind="ExternalInput").ap(),
        "out": nc.dram_tensor("out", [T, 5, D], F32,
                              kind="ExternalOutput").ap(),
    }
    with tile.TileContext(nc) as tc:
        build_tile_program(tc, io, T, tps)
    nc.compile()
    return nc


_NC_CACHE = {}


def kernel(x, state, time_maa_x, time_maa_w1, time_maa_w2,
           maa_k, maa_w, maa_v, maa_r, maa_g, i):
    global LAST_RESULT
    x = np.asarray(x, np.float32)
    state = np.asarray(state, np.float32)
    B, S, D_ = x.shape
    assert (B, S, D_) == (4, 2048, D)
    T = S * B // N_CORES  # 1024 tokens per core
    i1 = (2 + HEAD) * int(i) + 1

    consts = host_pack_weights(time_maa_x, time_maa_w1, time_maa_w2,
                               maa_k, maa_w, maa_v, maa_r, maa_g)
    in_maps = []
    for c in range(N_CORES):
        b, half = c // 2, c % 2
        xs = np.ascontiguousarray(x[b, half * T:(half + 1) * T])
        prev = state[b, i1] if half == 0 else x[b, T - 1]
        in_maps.append({
            "x": xs,
            "xbf": xs.astype(ml_dtypes.bfloat16),
            "halo0": np.ascontiguousarray(prev[None], np.float32),
            "haloT": np.ascontiguousarray(
                prev.reshape(NK, 128).T).astype(ml_dtypes.bfloat16),
            **consts})

    key = T
    if key not in _NC_CACHE:
        _NC_CACHE[key] = build_nc(T)
    nc = _NC_CACHE[key]

    global _LAST_IN_MAPS
    _LAST_IN_MAPS = in_maps
    LAST_RESULT = run_bass_kernel_spmd(nc, in_maps, list(range(N_CORES)))
    res = LAST_RESULT.results

    out = np.empty((B, S, 5, D), np.float32)
    for c in range(N_CORES):
        b, half = c // 2, c % 2
        out[b, half * T:(half + 1) * T] = res[c]["out"].reshape(T, 5, D)

    new_state = state.copy()
    new_state[:, i1] = x[:, -1]
    return out, new_state


# revision 37
# speedup vs baseline: 1.1168x; 1.1168x over previous
# RWKV token-shift + LoRA mixing block for Trainium2, 8-core SPMD.
#
# Reference computation (per batch b, token t):
#   sx[t]     = x[t-1] - x[t]            (x[-1] comes from state row i1)
#   xk        = x + sx * time_maa_x
#   h         = tanh(xk @ w1)            # [T, 160]
#   xxx[f]    = h[:, 32f:32f+32] @ w2[f] # [T, D] for f in 0..4
#   out[t,f]  = x[t] + sx[t] * (maa_f + xxx[t,f])
#   new_state = state with row i1 := x[:, -1]
#
# Sharding: 8 cores = (batch b = c//2) x (sequence half = c%2), 1024 tokens
# per core.  Each 128-token tile's halo (token t0-1) comes from row 127 of
# the previous x tile via a one-hot PE matmul; tile 0 uses a DRAM halo row.
#
# Device strategy per core (T=1024, D=2048):
#  - sx via PE:  psum = (S - I) @ x_tile + halo row (e0/e127 matmuls, all
#    float32r = full-rate fp32 bits), ACT copy to SBUF.  No cross-partition
#    shifts ever touch the vector engines.
#  - xT (bf16) is built by 16 single-shot DMA xbar transposes from a
#    host-cast bf16 copy of x; a padded halo column keeps x[t-1] reads as a
#    free-dim offset (token columns stay 32B-aligned).
#  - stage 1 (hT = tanh(xk @ w1)) uses the identity
#        xk @ w1 = x @ (w1 - tmx*w1) + x_prev @ (tmx*w1)
#    with both weight halves host-packed bf16: no elementwise prep at all.
#    Tiny selection matmuls route each f's 32 pre-tanh rows into a [33, N]
#    block whose 33rd row becomes exactly 1.0 via the tanh per-partition
#    bias (tanh(20) == 1), so stage 2 adds maa_f through the packed
#    [w2_f; maa_f] 33rd row in a single K=33 matmul.
#  - final: DVE  out = psum * sx  (only DVE can read PSUM for TT); the
#    "+ x" add is split DVE / GPSIMD / PE(identity-matmul accumulate with
#    ACT copy-out) to spread elementwise work across every engine, then
#    HWDGE-stores go straight into out[t, f, :] (stores on the ACT ring,
#    loads + transposes on the sync ring).
import sys

if "/opt/trn_rl_repo" not in sys.path:
    sys.path.insert(0, "/opt/trn_rl_repo")

import numpy as np
import ml_dtypes

import concourse.bass as bass
import concourse.mybir as mybir
from concourse import bacc, tile
from concourse.bass_utils import run_bass_kernel_spmd

F32 = mybir.dt.float32
F32R = mybir.dt.float32r  # fp32 bits, reduced-precision multiply, 4x faster PE
BF16 = mybir.dt.bfloat16
TANH = mybir.ActivationFunctionType.Tanh

D = 2048
J = 160
NK = D // 128  # 16 d-chunks
HEAD = 64
N_CORES = 8

LAST_RESULT = None  # BassKernelResults of the most recent run (for profiling)


def build_tile_program(tc, io, T, tps=2):
    """Emit the per-core tile program.

    io: dict name -> bass.AP for dram tensors.
    T: tokens handled by this core.  tps: 128-token tiles per super-tile.
    """
    from contextlib import ExitStack
    ctx = ExitStack()
    nc = tc.nc
    nt = T // 128
    assert nt % tps == 0
    N = tps * 128  # stage-1/2 token block width

    x_d, out_d = io["x"], io["out"]

    wts = ctx.enter_context(tc.tile_pool(name="wts", bufs=1))
    xp = ctx.enter_context(tc.tile_pool(name="xp", bufs=min(nt, 6)))
    sxp = ctx.enter_context(tc.tile_pool(name="sxp", bufs=min(nt, tps + 2)))
    xtp = ctx.enter_context(tc.tile_pool(name="xtp", bufs=NK))
    htp = ctx.enter_context(tc.tile_pool(name="htp", bufs=2))
    outp = ctx.enter_context(tc.tile_pool(name="outp", bufs=5))
    pss = ctx.enter_context(
        tc.tile_pool(name="pss", bufs=4, space=bass.MemorySpace.PSUM))
    pso = ctx.enter_context(
        tc.tile_pool(name="pso", bufs=4, space=bass.MemorySpace.PSUM))

    def load_const(name, shape, dtype):
        t = wts.tile(shape, dtype, tag=name, name=name + "_sb")
        nc.sync.dma_start(t[:], io[name][:].bitcast(dtype))
        return t

    wpack = load_const("wpack", [128, 2 * NK * J], BF16)
    selp = load_const("selp", [128, 4 * 33], F32R)
    sel4 = load_const("sel4", [32, 33], F32R)
    w2all = load_const("w2all", [33, 5 * D], BF16)
    shiftm = load_const("shiftm", [128, 128], F32R)
    e0 = load_const("e0", [1, 128], F32R)
    e127 = load_const("e127", [128, 128], F32R)
    halo0 = load_const("halo0", [1, D], F32R)
    biasv = load_const("biasv", [128, 1], F32)
    ident = load_const("ident", [128, 128], F32R)
    haloT = load_const("haloT", [128, NK], BF16)

    # Resident transposed x, one [128, 16+T] bf16 tile per 128-wide d-chunk.
    # Column 16+t is token t; column 15 is the halo token (t = -1); cols
    # 0-14 pad so token columns stay 32-byte aligned for the xbar DMA.
    # Each tile is filled by ONE DMA-transpose from the host-cast bf16 x.
    xT = [xtp.tile([128, 16 + T], BF16, tag="xT", name=f"xT{k}")
          for k in range(NK)]
    xbf_d = io["xbf"]
    for k in range(NK):
        nc.sync.dma_start(xT[k][:, 16:16 + T],
                          xbf_d[:, k * 128:(k + 1) * 128], transpose=True)
        nc.scalar.copy(xT[k][:, 15:16], haloT[:, k:k + 1])

    x_tiles = [None] * nt
    sx_tiles = [None] * nt

    for i in range(nt):
        xi = xp.tile([128, D], F32R, tag="x")
        nc.sync.dma_start(xi[:], x_d[i * 128:(i + 1) * 128, :].bitcast(F32R))
        x_tiles[i] = xi

        # sx = (S - I) @ x + halo into row 0   (PE), then ACT copy to SBUF.
        # Tile 0's halo is the DRAM halo row (rank-1 via e0); later tiles
        # take row 127 of the previous x tile (one-hot e127 matmul).
        sxi = sxp.tile([128, D], F32, tag="sx")
        for c in range(4):
            ps = pss.tile([128, 512], F32, tag="ps")
            cs = slice(c * 512, (c + 1) * 512)
            nc.tensor.matmul(ps[:], shiftm[:], xi[:, cs],
                             start=True, stop=False)
            if i == 0:
                nc.tensor.matmul(ps[:], e0[:], halo0[0:1, cs],
                                 start=False, stop=True)
            else:
                nc.tensor.matmul(ps[:], e127[:], x_tiles[i - 1][:, cs],
                                 start=False, stop=True)
            nc.scalar.copy(sxi[:, cs], ps[:])
        sx_tiles[i] = sxi

        if i % tps != tps - 1:
            continue

        # ---- super-tile s complete: stage 1 then stage 2 ----
        # Each f's 32 hidden units land at partition base 0 or 64 of one of
        # three ht tensors, followed by a memset ones-row at base+32 so
        # stage 2 is a single K=33 matmul that adds maa_f via the packed
        # w2 ([w2_f; maa_f]) 33rd row.  f0||f1 (f2||f3) run in concurrent
        # PE row/col groups.
        s = i // tps

        psH = pss.tile([128, N], F32, tag="ps")
        psH2 = pss.tile([32, N], F32, tag="ps")
        for (mo, msz, pt_) in ((0, 128, psH), (128, 32, psH2)):
            nmm = 0
            for pass_ in range(2):      # 0: w1b against x,  1: w1p against x_prev
                off = 16 - pass_        # halo-padded column offset
                for k in range(NK):
                    c0 = (pass_ * NK + k) * J + mo
                    nc.tensor.matmul(
                        pt_[:],
                        wpack[:, c0:c0 + msz],
                        xT[k][:, s * N + off: s * N + off + N],
                        start=(nmm == 0), stop=(nmm == 2 * NK - 1))
                    nmm += 1
        hpre = htp.tile([128, N], F32R, tag="hpre")
        hpre2 = htp.tile([32, N], F32R, tag="hpre2")
        nc.scalar.copy(hpre[:], psH[:])
        nc.scalar.copy(hpre2[:], psH2[:])

        # Route each f's 32 rows into a base-0 [33, N] block via a tiny
        # selection matmul, then tanh (+bias -> ones row at row 32).
        hts = []
        for f in range(5):
            ps_t = pss.tile([33, N], F32, tag="ps", name=f"psel{f}")
            ht_t = htp.tile([33, N], BF16, tag="ht", name=f"ht{f}",
                            bufs=10)
            if f < 4:
                nc.tensor.matmul(ps_t[:], selp[:, 33 * f:33 * f + 33],
                                 hpre[:], start=True, stop=True)
            else:
                nc.tensor.matmul(ps_t[:], sel4[:], hpre2[:],
                                 start=True, stop=True)
            nc.scalar.activation(ht_t[:], ps_t[:], TANH,
                                 bias=biasv[0:33, 0:1])
            hts.append(ht_t)

        for ii in range(s * tps, (s + 1) * tps):
            tloc = (ii % tps) * 128
            outs = [outp.tile([128, D], F32, tag="out", name=f"out{ii}_{f}")
                    for f in range(5)]
            # Final combine route per (ii, f): the "+ x" add runs on DVE,
            # GPSIMD, or the PE (identity-matmul accumulate onto the
            # DVE-scaled psum, then ACT copies PSUM->SBUF) to spread the
            # elementwise work across every engine.
            rr = {0: 0, 1: 3, 2: 6, 3: 3, 4: 6, 5: 0, 6: 3, 7: 6}
            route = {f: rr[(ii * 5 + f) % 8] for f in range(5)}
            for f in range(5):
                for c in range(4):
                    cs = slice(c * 512, (c + 1) * 512)
                    # 1-bank psum chunks + 4-deep pool: the PE can run four
                    # chunks ahead of the DVE drain instead of one tile.
                    pc = pso.tile([128, 512], F32, tag="po", name=f"po{f}_{c}")
                    nc.tensor.matmul(
                        pc[:],
                        hts[f][:, tloc:tloc + 128],
                        w2all[:, f * D + c * 512: f * D + (c + 1) * 512],
                        start=True, stop=True)
                    if route[f] < 6:
                        nc.vector.tensor_mul(outs[f][:, cs], pc[:],
                                             sx_tiles[ii][:, cs])
                    else:
                        # scale in place, accumulate x on PE, copy out
                        nc.vector.tensor_mul(pc[:], pc[:],
                                             sx_tiles[ii][:, cs])
                        nc.tensor.matmul(pc[:], ident[:], x_tiles[ii][:, cs],
                                         start=False, stop=True,
                                         skip_group_check=True)
                        nc.scalar.copy(outs[f][:, cs], pc[:])
            for f in range(5):
                if route[f] < 6:
                    eng = nc.vector if route[f] < 2 else nc.gpsimd
                    eng.tensor_add(outs[f][:], outs[f][:],
                                   x_tiles[ii][:].bitcast(F32))
                # Stores ride the ACT HWDGE ring; loads/transposes use the
                # sync ring — two rings double the DMA issue bandwidth.
                nc.scalar.dma_start(out_d[ii * 128:(ii + 1) * 128, f, :],
                                    outs[f][:])
    ctx.close()


def host_pack_weights(time_maa_x, time_maa_w1, time_maa_w2,
                      maa_k, maa_w, maa_v, maa_r, maa_g):
    tmx = np.asarray(time_maa_x, np.float32)
    w1 = np.asarray(time_maa_w1, np.float32)
    w2 = np.asarray(time_maa_w2, np.float32)
    w1p = w1 * tmx[:, None]
    w1b = w1 - w1p
    # wpack[p, pass, k, j] = w1x[pass][k*128+p, j]
    wpack = (np.stack([w1b, w1p], 0)
             .reshape(2, NK, 128, J)
             .transpose(2, 0, 1, 3)
             .reshape(128, 2 * NK * J)
             .astype(ml_dtypes.bfloat16))
    # Selection matrices: route pre-tanh h rows 32f..32f+31 into a 33-row
    # block (33rd col zero; the tanh bias writes the ones row).
    selp = np.zeros((128, 4 * 33), np.float32)
    for f in range(4):
        for m in range(32):
            selp[32 * f + m, 33 * f + m] = 1.0
    sel4 = np.zeros((32, 33), np.float32)
    sel4[:32, :32] = np.eye(32)
    maas = np.stack([maa_k, maa_w, maa_v, maa_r, maa_g]).astype(np.float32)
    e127 = np.zeros((128, 128), np.float32)
    e127[127, 0] = 1.0
    biasv = np.zeros((128, 1), np.float32)
    biasv[[32, 96]] = 20.0  # tanh(20) == 1.0 -> ones row for K=33 matmuls

    w2all = np.zeros((33, 5 * D), np.float32)
    for f in range(5):
        w2all[0:32, f * D:(f + 1) * D] = w2[f]
        w2all[32, f * D:(f + 1) * D] = maas[f]

    consts = {
        "wpack": wpack,
        "w2all": w2all.astype(ml_dtypes.bfloat16),
        "shiftm": (np.eye(128, 128, 1) - np.eye(128)).astype(np.float32),
        "e0": np.eye(1, 128, dtype=np.float32),
        "e127": e127,
        "biasv": biasv,
        "ident": np.eye(128, dtype=np.float32),
        "selp": selp,
        "sel4": sel4,
    }
    return consts


def build_nc(T, tps=2):
    # Bacc (not bare Bass): its compile() runs move_matmul_waits_to_ldweights
    # and generate_event_semaphores, which split multi-semaphore waits to
    # satisfy the 1-wait-per-instruction TRN2 constraint walrus enforces.
    nc = bacc.Bacc("TRN2", target_bir_lowering=False, debug=False)
    nt = T // 128
    io = {
        "x": nc.dram_tensor("x", [T, D], F32, kind="ExternalInput").ap(),
        "xbf": nc.dram_tensor("xbf", [T, D], BF16, kind="ExternalInput").ap(),
        "halo0": nc.dram_tensor("halo0", [1, D], F32,
                                kind="ExternalInput").ap(),
        "haloT": nc.dram_tensor("haloT", [128, NK], BF16,
                                kind="ExternalInput").ap(),
        "wpack": nc.dram_tensor("wpack", [128, 2 * NK * J], BF16,
                                kind="ExternalInput").ap(),
        "selp": nc.dram_tensor("selp", [128, 4 * 33], F32,
                               kind="ExternalInput").ap(),
        "sel4": nc.dram_tensor("sel4", [32, 33], F32,
                               kind="ExternalInput").ap(),
        "w2all": nc.dram_tensor("w2all", [33, 5 * D], BF16,
                                kind="ExternalInput").ap(),
        "shiftm": nc.dram_tensor("shiftm", [128, 128], F32,
                                 kind="ExternalInput").ap(),
        "e0": nc.dram_tensor("e0", [1, 128], F32, kind="ExternalInput").ap(),
        "e127": nc.dram_tensor("e127", [128, 128], F32,
                               kind="ExternalInput").ap(),
        "biasv": nc.dram_tensor("biasv", [128, 1], F32,
                                kind="ExternalInput").ap(),
        "ident": nc.dram_tensor("ident", [128, 128], F32,
                                kind="ExternalInput").ap(),
        "out": nc.dram_tensor("out", [T, 5, D], F32,
                              kind="ExternalOutput").ap(),
    }
    with tile.TileContext(nc) as tc:
        build_tile_program(tc, io, T, tps)
    nc.compile()
    return nc


_NC_CACHE = {}


def kernel(x, state, time_maa_x, time_maa_w1, time_maa_w2,
           maa_k, maa_w, maa_v, maa_r, maa_g, i):
    global LAST_RESULT
    x = np.asarray(x, np.float32)
    state = np.asarray(state, np.float32)
    B, S, D_ = x.shape
    assert (B, S, D_) == (4, 2048, D)
    T = S * B // N_CORES  # 1024 tokens per core
    i1 = (2 + HEAD) * int(i) + 1

    consts = host_pack_weights(time_maa_x, time_maa_w1, time_maa_w2,
                               maa_k, maa_w, maa_v, maa_r, maa_g)
    in_maps = []
    for c in range(N_CORES):
        b, half = c // 2, c % 2
        xs = np.ascontiguousarray(x[b, half * T:(half + 1) * T])
        prev = state[b, i1] if half == 0 else x[b, T - 1]
        in_maps.append({
            "x": xs,
            "xbf": xs.astype(ml_dtypes.bfloat16),
            "halo0": np.ascontiguousarray(prev[None], np.float32),
            "haloT": np.ascontiguousarray(
                prev.reshape(NK, 128).T).astype(ml_dtypes.bfloat16),
            **consts})

    key = T
    if key not in _NC_CACHE:
        _NC_CACHE[key] = build_nc(T)
    nc = _NC_CACHE[key]

    global _LAST_IN_MAPS
    _LAST_IN_MAPS = in_maps
    LAST_RESULT = run_bass_kernel_spmd(nc, in_maps, list(range(N_CORES)))
    res = LAST_RESULT.results

    out = np.empty((B, S, 5, D), np.float32)
    for c in range(N_CORES):
        b, half = c // 2, c % 2
        out[b, half * T:(half + 1) * T] = res[c]["out"].reshape(T, 5, D)

    new_state = state.copy()
    new_state[:, i1] = x[:, -1]
    return out, new_state


# revision 38
# speedup vs baseline: 1.1824x; 1.0587x over previous
# RWKV token-shift + LoRA mixing block for Trainium2, 8-core SPMD.
#
# Reference computation (per batch b, token t):
#   sx[t]     = x[t-1] - x[t]            (x[-1] comes from state row i1)
#   xk        = x + sx * time_maa_x
#   h         = tanh(xk @ w1)            # [T, 160]
#   xxx[f]    = h[:, 32f:32f+32] @ w2[f] # [T, D] for f in 0..4
#   out[t,f]  = x[t] + sx[t] * (maa_f + xxx[t,f])
#   new_state = state with row i1 := x[:, -1]
#
# Sharding: 8 cores = (batch b = c//2) x (sequence half = c%2), 1024 tokens
# per core.  Each 128-token tile's halo (token t0-1) comes from row 127 of
# the previous x tile via a one-hot PE matmul; tile 0 uses a DRAM halo row.
#
# Device strategy per core (T=1024, D=2048):
#  - sx via PE:  psum = (S - I) @ x_tile + halo row (e0/e127 matmuls, all
#    float32r = full-rate fp32 bits), ACT copy to SBUF.  No cross-partition
#    shifts ever touch the vector engines.
#  - xT (bf16) is built by 16 single-shot DMA xbar transposes from a
#    host-cast bf16 copy of x; a padded halo column keeps x[t-1] reads as a
#    free-dim offset (token columns stay 32B-aligned).
#  - stage 1 (hT = tanh(xk @ w1)) uses the identity
#        xk @ w1 = x @ (w1 - tmx*w1) + x_prev @ (tmx*w1)
#    with both weight halves host-packed bf16: no elementwise prep at all.
#    Tiny selection matmuls route each f's 32 pre-tanh rows into a [33, N]
#    block whose 33rd row becomes exactly 1.0 via the tanh per-partition
#    bias (tanh(20) == 1), so stage 2 adds maa_f through the packed
#    [w2_f; maa_f] 33rd row in a single K=33 matmul.
#  - final: DVE  out = psum * sx  (only DVE can read PSUM for TT); the
#    "+ x" add is split DVE / GPSIMD / PE(identity-matmul accumulate with
#    ACT copy-out) to spread elementwise work across every engine, then
#    HWDGE-stores go straight into out[t, f, :] (stores on the ACT ring,
#    loads + transposes on the sync ring).
import sys

if "/opt/trn_rl_repo" not in sys.path:
    sys.path.insert(0, "/opt/trn_rl_repo")

import numpy as np
import ml_dtypes

import concourse.bass as bass
import concourse.mybir as mybir
from concourse import bacc, tile
from concourse.bass_utils import run_bass_kernel_spmd

F32 = mybir.dt.float32
F32R = mybir.dt.float32r  # fp32 bits, reduced-precision multiply, 4x faster PE
BF16 = mybir.dt.bfloat16
TANH = mybir.ActivationFunctionType.Tanh

D = 2048
J = 160
NK = D // 128  # 16 d-chunks
HEAD = 64
N_CORES = 8

LAST_RESULT = None  # BassKernelResults of the most recent run (for profiling)


def build_tile_program(tc, io, T, tps=2):
    """Emit the per-core tile program.

    io: dict name -> bass.AP for dram tensors.
    T: tokens handled by this core.  tps: 128-token tiles per super-tile.
    """
    from contextlib import ExitStack
    ctx = ExitStack()
    nc = tc.nc
    nt = T // 128
    assert nt % tps == 0
    N = tps * 128  # stage-1/2 token block width

    x_d, out_d = io["x"], io["out"]

    wts = ctx.enter_context(tc.tile_pool(name="wts", bufs=1))
    xp = ctx.enter_context(tc.tile_pool(name="xp", bufs=min(nt, 6)))
    sxp = ctx.enter_context(tc.tile_pool(name="sxp", bufs=min(nt, tps + 1)))
    xpp = ctx.enter_context(tc.tile_pool(name="xpp", bufs=2))
    xtp = ctx.enter_context(tc.tile_pool(name="xtp", bufs=NK))
    htp = ctx.enter_context(tc.tile_pool(name="htp", bufs=2))
    outp = ctx.enter_context(tc.tile_pool(name="outp", bufs=5))
    pss = ctx.enter_context(
        tc.tile_pool(name="pss", bufs=4, space=bass.MemorySpace.PSUM))
    pso = ctx.enter_context(
        tc.tile_pool(name="pso", bufs=4, space=bass.MemorySpace.PSUM))

    def load_const(name, shape, dtype):
        t = wts.tile(shape, dtype, tag=name, name=name + "_sb")
        nc.sync.dma_start(t[:], io[name][:].bitcast(dtype))
        return t

    wpack = load_const("wpack", [128, 2 * NK * J], BF16)
    selp = load_const("selp", [128, 4 * 33], F32R)
    sel4 = load_const("sel4", [32, 33], F32R)
    w2all = load_const("w2all", [33, 5 * D], BF16)
    biasv = load_const("biasv", [128, 1], F32)
    ident = load_const("ident", [128, 128], F32R)
    haloT = load_const("haloT", [128, NK], BF16)

    # Resident transposed x, one [128, 16+T] bf16 tile per 128-wide d-chunk.
    # Column 16+t is token t; column 15 is the halo token (t = -1); cols
    # 0-14 pad so token columns stay 32-byte aligned for the xbar DMA.
    # Each tile is filled by ONE DMA-transpose from the host-cast bf16 x.
    xT = [xtp.tile([128, 16 + T], BF16, tag="xT", name=f"xT{k}")
          for k in range(NK)]
    xbf_d = io["xbf"]
    for k in range(NK):
        nc.sync.dma_start(xT[k][:, 16:16 + T],
                          xbf_d[:, k * 128:(k + 1) * 128], transpose=True)
        nc.scalar.copy(xT[k][:, 15:16], haloT[:, k:k + 1])

    x_tiles = [None] * nt
    sx_tiles = [None] * nt

    for i in range(nt):
        xi = xp.tile([128, D], F32R, tag="x")
        nc.sync.dma_start(xi[:],
                          x_d[1 + i * 128:1 + (i + 1) * 128, :].bitcast(F32R))
        x_tiles[i] = xi

        # x arrives with the halo row prepended, so x_prev tiles are plain
        # DRAM loads and sx = x_prev - x runs on GPSIMD (which has queue
        # slack) -- no PE or ACT work at all for the token shift.
        sxi = sxp.tile([128, D], F32, tag="sx")
        for hlf in range(2):
            hs = slice(hlf * 1024, (hlf + 1) * 1024)
            xpv = xpp.tile([128, 1024], F32, tag="xpv")
            nc.sync.dma_start(xpv[:], x_d[i * 128:(i + 1) * 128, hs])
            nc.gpsimd.tensor_sub(sxi[:, hs], xpv[:], xi[:, hs].bitcast(F32))
        sx_tiles[i] = sxi

        if i % tps != tps - 1:
            continue

        # ---- super-tile s complete: stage 1 then stage 2 ----
        # Each f's 32 hidden units land at partition base 0 or 64 of one of
        # three ht tensors, followed by a memset ones-row at base+32 so
        # stage 2 is a single K=33 matmul that adds maa_f via the packed
        # w2 ([w2_f; maa_f]) 33rd row.  f0||f1 (f2||f3) run in concurrent
        # PE row/col groups.
        s = i // tps

        psH = pss.tile([128, N], F32, tag="ps")
        psH2 = pss.tile([32, N], F32, tag="ps")
        for (mo, msz, pt_) in ((0, 128, psH), (128, 32, psH2)):
            nmm = 0
            for pass_ in range(2):      # 0: w1b against x,  1: w1p against x_prev
                off = 16 - pass_        # halo-padded column offset
                for k in range(NK):
                    c0 = (pass_ * NK + k) * J + mo
                    nc.tensor.matmul(
                        pt_[:],
                        wpack[:, c0:c0 + msz],
                        xT[k][:, s * N + off: s * N + off + N],
                        start=(nmm == 0), stop=(nmm == 2 * NK - 1))
                    nmm += 1
        hpre = htp.tile([128, N], F32R, tag="hpre")
        hpre2 = htp.tile([32, N], F32R, tag="hpre2")
        nc.scalar.copy(hpre[:], psH[:])
        nc.scalar.copy(hpre2[:], psH2[:])

        # Route each f's 32 rows into a base-0 [33, N] block via a tiny
        # selection matmul, then tanh (+bias -> ones row at row 32).
        hts = []
        for f in range(5):
            ps_t = pss.tile([33, N], F32, tag="ps", name=f"psel{f}")
            ht_t = htp.tile([33, N], BF16, tag="ht", name=f"ht{f}",
                            bufs=10)
            if f < 4:
                nc.tensor.matmul(ps_t[:], selp[:, 33 * f:33 * f + 33],
                                 hpre[:], start=True, stop=True)
            else:
                nc.tensor.matmul(ps_t[:], sel4[:], hpre2[:],
                                 start=True, stop=True)
            nc.scalar.activation(ht_t[:], ps_t[:], TANH,
                                 bias=biasv[0:33, 0:1])
            hts.append(ht_t)

        for ii in range(s * tps, (s + 1) * tps):
            tloc = (ii % tps) * 128
            outs = [outp.tile([128, D], F32, tag="out", name=f"out{ii}_{f}")
                    for f in range(5)]
            # Final combine route per (ii, f): the "+ x" add runs on DVE,
            # GPSIMD, or the PE (identity-matmul accumulate onto the
            # DVE-scaled psum, then ACT copies PSUM->SBUF) to spread the
            # elementwise work across every engine.
            rr = {0: 0, 1: 3, 2: 6, 3: 3, 4: 6, 5: 0, 6: 3, 7: 6}
            route = {f: rr[(ii * 5 + f) % 8] for f in range(5)}
            for f in range(5):
                for c in range(4):
                    cs = slice(c * 512, (c + 1) * 512)
                    # 1-bank psum chunks + 4-deep pool: the PE can run four
                    # chunks ahead of the DVE drain instead of one tile.
                    pc = pso.tile([128, 512], F32, tag="po", name=f"po{f}_{c}")
                    nc.tensor.matmul(
                        pc[:],
                        hts[f][:, tloc:tloc + 128],
                        w2all[:, f * D + c * 512: f * D + (c + 1) * 512],
                        start=True, stop=True)
                    if route[f] < 6:
                        nc.vector.tensor_mul(outs[f][:, cs], pc[:],
                                             sx_tiles[ii][:, cs])
                    else:
                        # scale in place, accumulate x on PE, copy out
                        nc.vector.tensor_mul(pc[:], pc[:],
                                             sx_tiles[ii][:, cs])
                        nc.tensor.matmul(pc[:], ident[:], x_tiles[ii][:, cs],
                                         start=False, stop=True,
                                         skip_group_check=True)
                        nc.scalar.copy(outs[f][:, cs], pc[:])
            for f in range(5):
                if route[f] < 6:
                    eng = nc.vector if route[f] < 2 else nc.gpsimd
                    eng.tensor_add(outs[f][:], outs[f][:],
                                   x_tiles[ii][:].bitcast(F32))
                # Stores ride the ACT HWDGE ring; loads/transposes use the
                # sync ring — two rings double the DMA issue bandwidth.
                nc.scalar.dma_start(out_d[ii * 128:(ii + 1) * 128, f, :],
                                    outs[f][:])
    ctx.close()


def host_pack_weights(time_maa_x, time_maa_w1, time_maa_w2,
                      maa_k, maa_w, maa_v, maa_r, maa_g):
    tmx = np.asarray(time_maa_x, np.float32)
    w1 = np.asarray(time_maa_w1, np.float32)
    w2 = np.asarray(time_maa_w2, np.float32)
    w1p = w1 * tmx[:, None]
    w1b = w1 - w1p
    # wpack[p, pass, k, j] = w1x[pass][k*128+p, j]
    wpack = (np.stack([w1b, w1p], 0)
             .reshape(2, NK, 128, J)
             .transpose(2, 0, 1, 3)
             .reshape(128, 2 * NK * J)
             .astype(ml_dtypes.bfloat16))
    # Selection matrices: route pre-tanh h rows 32f..32f+31 into a 33-row
    # block (33rd col zero; the tanh bias writes the ones row).
    selp = np.zeros((128, 4 * 33), np.float32)
    for f in range(4):
        for m in range(32):
            selp[32 * f + m, 33 * f + m] = 1.0
    sel4 = np.zeros((32, 33), np.float32)
    sel4[:32, :32] = np.eye(32)
    maas = np.stack([maa_k, maa_w, maa_v, maa_r, maa_g]).astype(np.float32)
    biasv = np.zeros((128, 1), np.float32)
    biasv[[32, 96]] = 20.0  # tanh(20) == 1.0 -> ones row for K=33 matmuls

    w2all = np.zeros((33, 5 * D), np.float32)
    for f in range(5):
        w2all[0:32, f * D:(f + 1) * D] = w2[f]
        w2all[32, f * D:(f + 1) * D] = maas[f]

    consts = {
        "wpack": wpack,
        "w2all": w2all.astype(ml_dtypes.bfloat16),

        "biasv": biasv,
        "ident": np.eye(128, dtype=np.float32),
        "selp": selp,
        "sel4": sel4,
    }
    return consts


def build_nc(T, tps=2):
    # Bacc (not bare Bass): its compile() runs move_matmul_waits_to_ldweights
    # and generate_event_semaphores, which split multi-semaphore waits to
    # satisfy the 1-wait-per-instruction TRN2 constraint walrus enforces.
    nc = bacc.Bacc("TRN2", target_bir_lowering=False, debug=False)
    nt = T // 128
    io = {
        "x": nc.dram_tensor("x", [1 + T, D], F32,
                    kind="ExternalInput").ap(),
        "xbf": nc.dram_tensor("xbf", [T, D], BF16, kind="ExternalInput").ap(),

        "haloT": nc.dram_tensor("haloT", [128, NK], BF16,
                                kind="ExternalInput").ap(),
        "wpack": nc.dram_tensor("wpack", [128, 2 * NK * J], BF16,
                                kind="ExternalInput").ap(),
        "selp": nc.dram_tensor("selp", [128, 4 * 33], F32,
                               kind="ExternalInput").ap(),
        "sel4": nc.dram_tensor("sel4", [32, 33], F32,
                               kind="ExternalInput").ap(),
        "w2all": nc.dram_tensor("w2all", [33, 5 * D], BF16,
                                kind="ExternalInput").ap(),

        "biasv": nc.dram_tensor("biasv", [128, 1], F32,
                                kind="ExternalInput").ap(),
        "ident": nc.dram_tensor("ident", [128, 128], F32,
                                kind="ExternalInput").ap(),
        "out": nc.dram_tensor("out", [T, 5, D], F32,
                              kind="ExternalOutput").ap(),
    }
    with tile.TileContext(nc) as tc:
        build_tile_program(tc, io, T, tps)
    nc.compile()
    return nc


_NC_CACHE = {}


def kernel(x, state, time_maa_x, time_maa_w1, time_maa_w2,
           maa_k, maa_w, maa_v, maa_r, maa_g, i):
    global LAST_RESULT
    x = np.asarray(x, np.float32)
    state = np.asarray(state, np.float32)
    B, S, D_ = x.shape
    assert (B, S, D_) == (4, 2048, D)
    T = S * B // N_CORES  # 1024 tokens per core
    i1 = (2 + HEAD) * int(i) + 1

    consts = host_pack_weights(time_maa_x, time_maa_w1, time_maa_w2,
                               maa_k, maa_w, maa_v, maa_r, maa_g)
    in_maps = []
    for c in range(N_CORES):
        b, half = c // 2, c % 2
        xs = np.ascontiguousarray(x[b, half * T:(half + 1) * T])
        prev = state[b, i1] if half == 0 else x[b, T - 1]
        in_maps.append({
            "x": np.concatenate([prev[None], xs], 0),
            "xbf": xs.astype(ml_dtypes.bfloat16),
            "haloT": np.ascontiguousarray(
                prev.reshape(NK, 128).T).astype(ml_dtypes.bfloat16),
            **consts})

    key = T
    if key not in _NC_CACHE:
        _NC_CACHE[key] = build_nc(T)
    nc = _NC_CACHE[key]

    global _LAST_IN_MAPS
    _LAST_IN_MAPS = in_maps
    LAST_RESULT = run_bass_kernel_spmd(nc, in_maps, list(range(N_CORES)))
    res = LAST_RESULT.results

    out = np.empty((B, S, 5, D), np.float32)
    for c in range(N_CORES):
        b, half = c // 2, c % 2
        out[b, half * T:(half + 1) * T] = res[c]["out"].reshape(T, 5, D)

    new_state = state.copy()
    new_state[:, i1] = x[:, -1]
    return out, new_state


# revision 39
# speedup vs baseline: 1.2414x; 1.0499x over previous
# RWKV token-shift + LoRA mixing block for Trainium2, 8-core SPMD.
#
# Reference computation (per batch b, token t):
#   sx[t]     = x[t-1] - x[t]            (x[-1] comes from state row i1)
#   xk        = x + sx * time_maa_x
#   h         = tanh(xk @ w1)            # [T, 160]
#   xxx[f]    = h[:, 32f:32f+32] @ w2[f] # [T, D] for f in 0..4
#   out[t,f]  = x[t] + sx[t] * (maa_f + xxx[t,f])
#   new_state = state with row i1 := x[:, -1]
#
# Sharding: 8 cores = (batch b = c//2) x (sequence half = c%2), 1024 tokens
# per core.  Each 128-token tile's halo (token t0-1) comes from row 127 of
# the previous x tile via a one-hot PE matmul; tile 0 uses a DRAM halo row.
#
# Device strategy per core (T=1024, D=2048):
#  - sx via PE:  psum = (S - I) @ x_tile + halo row (e0/e127 matmuls, all
#    float32r = full-rate fp32 bits), ACT copy to SBUF.  No cross-partition
#    shifts ever touch the vector engines.
#  - xT (bf16) is built by 16 single-shot DMA xbar transposes from a
#    host-cast bf16 copy of x; a padded halo column keeps x[t-1] reads as a
#    free-dim offset (token columns stay 32B-aligned).
#  - stage 1 (hT = tanh(xk @ w1)) uses the identity
#        xk @ w1 = x @ (w1 - tmx*w1) + x_prev @ (tmx*w1)
#    with both weight halves host-packed bf16: no elementwise prep at all.
#    Tiny selection matmuls route each f's 32 pre-tanh rows into a [33, N]
#    block whose 33rd row becomes exactly 1.0 via the tanh per-partition
#    bias (tanh(20) == 1), so stage 2 adds maa_f through the packed
#    [w2_f; maa_f] 33rd row in a single K=33 matmul.
#  - final: DVE  out = psum * sx  (only DVE can read PSUM for TT); the
#    "+ x" add is split DVE / GPSIMD / PE(identity-matmul accumulate with
#    ACT copy-out) to spread elementwise work across every engine, then
#    HWDGE-stores go straight into out[t, f, :] (stores on the ACT ring,
#    loads + transposes on the sync ring).
import sys

if "/opt/trn_rl_repo" not in sys.path:
    sys.path.insert(0, "/opt/trn_rl_repo")

import numpy as np
import ml_dtypes

import concourse.bass as bass
import concourse.mybir as mybir
from concourse import bacc, tile
from concourse.bass_utils import run_bass_kernel_spmd

F32 = mybir.dt.float32
F32R = mybir.dt.float32r  # fp32 bits, reduced-precision multiply, 4x faster PE
BF16 = mybir.dt.bfloat16
TANH = mybir.ActivationFunctionType.Tanh

D = 2048
J = 160
NK = D // 128  # 16 d-chunks
HEAD = 64
N_CORES = 8

LAST_RESULT = None  # BassKernelResults of the most recent run (for profiling)


def build_tile_program(tc, io, T, tps=4):
    """Emit the per-core tile program.

    io: dict name -> bass.AP for dram tensors.
    T: tokens handled by this core.  tps: 128-token tiles per super-tile.
    """
    from contextlib import ExitStack
    ctx = ExitStack()
    nc = tc.nc
    nt = T // 128
    assert nt % tps == 0
    N = tps * 128  # stage-1/2 token block width

    x_d, out_d = io["x"], io["out"]

    wts = ctx.enter_context(tc.tile_pool(name="wts", bufs=1))
    xp = ctx.enter_context(tc.tile_pool(name="xp", bufs=min(nt, 5)))
    sxp = ctx.enter_context(tc.tile_pool(name="sxp", bufs=min(nt, tps + 1)))
    xpp = ctx.enter_context(tc.tile_pool(name="xpp", bufs=2))
    xtp = ctx.enter_context(tc.tile_pool(name="xtp", bufs=NK))
    htp = ctx.enter_context(tc.tile_pool(name="htp", bufs=2))
    outp = ctx.enter_context(tc.tile_pool(name="outp", bufs=5))
    pss = ctx.enter_context(
        tc.tile_pool(name="pss", bufs=4, space=bass.MemorySpace.PSUM))
    pso = ctx.enter_context(
        tc.tile_pool(name="pso", bufs=4, space=bass.MemorySpace.PSUM))

    def load_const(name, shape, dtype):
        t = wts.tile(shape, dtype, tag=name, name=name + "_sb")
        nc.sync.dma_start(t[:], io[name][:].bitcast(dtype))
        return t

    wpack = load_const("wpack", [128, 2 * NK * J], BF16)
    selp = load_const("selp", [128, 4 * 33], F32R)
    sel4 = load_const("sel4", [32, 33], F32R)
    w2all = load_const("w2all", [33, 5 * D], BF16)
    biasv = load_const("biasv", [128, 1], F32)
    ident = load_const("ident", [128, 128], F32R)
    haloT = load_const("haloT", [128, NK], BF16)

    # Resident transposed x, one [128, 16+T] bf16 tile per 128-wide d-chunk.
    # Column 16+t is token t; column 15 is the halo token (t = -1); cols
    # 0-14 pad so token columns stay 32-byte aligned for the xbar DMA.
    # Each tile is filled by ONE DMA-transpose from the host-cast bf16 x.
    xT = [xtp.tile([128, 16 + T], BF16, tag="xT", name=f"xT{k}")
          for k in range(NK)]
    xbf_d = io["xbf"]
    for k in range(NK):
        nc.sync.dma_start(xT[k][:, 16:16 + T],
                          xbf_d[:, k * 128:(k + 1) * 128], transpose=True)
        nc.scalar.copy(xT[k][:, 15:16], haloT[:, k:k + 1])

    x_tiles = [None] * nt
    sx_tiles = [None] * nt

    for i in range(nt):
        xi = xp.tile([128, D], F32R, tag="x")
        nc.sync.dma_start(xi[:],
                          x_d[1 + i * 128:1 + (i + 1) * 128, :].bitcast(F32R))
        x_tiles[i] = xi

        # x arrives with the halo row prepended, so x_prev tiles are plain
        # DRAM loads and sx = x_prev - x runs on GPSIMD (which has queue
        # slack) -- no PE or ACT work at all for the token shift.
        sxi = sxp.tile([128, D], F32, tag="sx")
        for hlf in range(2):
            hs = slice(hlf * 1024, (hlf + 1) * 1024)
            xpv = xpp.tile([128, 1024], F32, tag="xpv")
            nc.sync.dma_start(xpv[:], x_d[i * 128:(i + 1) * 128, hs])
            nc.gpsimd.tensor_sub(sxi[:, hs], xpv[:], xi[:, hs].bitcast(F32))
        sx_tiles[i] = sxi

        if i % tps != tps - 1:
            continue

        # ---- super-tile s complete: stage 1 then stage 2 ----
        # Each f's 32 hidden units land at partition base 0 or 64 of one of
        # three ht tensors, followed by a memset ones-row at base+32 so
        # stage 2 is a single K=33 matmul that adds maa_f via the packed
        # w2 ([w2_f; maa_f]) 33rd row.  f0||f1 (f2||f3) run in concurrent
        # PE row/col groups.
        s = i // tps

        psH = pss.tile([128, N], F32, tag="ps")
        psH2 = pss.tile([32, N], F32, tag="ps")
        for (mo, msz, pt_) in ((0, 128, psH), (128, 32, psH2)):
            nmm = 0
            for pass_ in range(2):      # 0: w1b against x,  1: w1p against x_prev
                off = 16 - pass_        # halo-padded column offset
                for k in range(NK):
                    c0 = (pass_ * NK + k) * J + mo
                    nc.tensor.matmul(
                        pt_[:],
                        wpack[:, c0:c0 + msz],
                        xT[k][:, s * N + off: s * N + off + N],
                        start=(nmm == 0), stop=(nmm == 2 * NK - 1))
                    nmm += 1
        hpre = htp.tile([128, N], F32R, tag="hpre")
        hpre2 = htp.tile([32, N], F32R, tag="hpre2")
        nc.scalar.copy(hpre[:], psH[:])
        nc.scalar.copy(hpre2[:], psH2[:])

        # Route each f's 32 rows into a base-0 [33, N] block via a tiny
        # selection matmul, then tanh (+bias -> ones row at row 32).
        hts = []
        for f in range(5):
            ps_t = pss.tile([33, N], F32, tag="ps", name=f"psel{f}")
            ht_t = htp.tile([33, N], BF16, tag="ht", name=f"ht{f}",
                            bufs=6)
            if f < 4:
                nc.tensor.matmul(ps_t[:], selp[:, 33 * f:33 * f + 33],
                                 hpre[:], start=True, stop=True)
            else:
                nc.tensor.matmul(ps_t[:], sel4[:], hpre2[:],
                                 start=True, stop=True)
            nc.scalar.activation(ht_t[:], ps_t[:], TANH,
                                 bias=biasv[0:33, 0:1])
            hts.append(ht_t)

        for ii in range(s * tps, (s + 1) * tps):
            tloc = (ii % tps) * 128
            outs = [outp.tile([128, D], F32, tag="out", name=f"out{ii}_{f}")
                    for f in range(5)]
            # Final combine route per (ii, f): the "+ x" add runs on DVE,
            # GPSIMD, or the PE (identity-matmul accumulate onto the
            # DVE-scaled psum, then ACT copies PSUM->SBUF) to spread the
            # elementwise work across every engine.
            rr = {0: 0, 1: 3, 2: 6, 3: 3, 4: 6, 5: 0, 6: 3, 7: 6}
            route = {f: rr[(ii * 5 + f) % 8] for f in range(5)}
            for f in range(5):
                for c in range(4):
                    cs = slice(c * 512, (c + 1) * 512)
                    # 1-bank psum chunks + 4-deep pool: the PE can run four
                    # chunks ahead of the DVE drain instead of one tile.
                    pc = pso.tile([128, 512], F32, tag="po", name=f"po{f}_{c}")
                    nc.tensor.matmul(
                        pc[:],
                        hts[f][:, tloc:tloc + 128],
                        w2all[:, f * D + c * 512: f * D + (c + 1) * 512],
                        start=True, stop=True)
                    if route[f] < 6:
                        nc.vector.tensor_mul(outs[f][:, cs], pc[:],
                                             sx_tiles[ii][:, cs])
                    else:
                        # scale in place, accumulate x on PE, copy out
                        nc.vector.tensor_mul(pc[:], pc[:],
                                             sx_tiles[ii][:, cs])
                        nc.tensor.matmul(pc[:], ident[:], x_tiles[ii][:, cs],
                                         start=False, stop=True,
                                         skip_group_check=True)
                        nc.scalar.copy(outs[f][:, cs], pc[:])
            for f in range(5):
                if route[f] < 6:
                    eng = nc.vector if route[f] < 2 else nc.gpsimd
                    eng.tensor_add(outs[f][:], outs[f][:],
                                   x_tiles[ii][:].bitcast(F32))
                # Stores ride the ACT HWDGE ring; loads/transposes use the
                # sync ring — two rings double the DMA issue bandwidth.
                nc.scalar.dma_start(out_d[ii * 128:(ii + 1) * 128, f, :],
                                    outs[f][:])
    ctx.close()


def host_pack_weights(time_maa_x, time_maa_w1, time_maa_w2,
                      maa_k, maa_w, maa_v, maa_r, maa_g):
    tmx = np.asarray(time_maa_x, np.float32)
    w1 = np.asarray(time_maa_w1, np.float32)
    w2 = np.asarray(time_maa_w2, np.float32)
    w1p = w1 * tmx[:, None]
    w1b = w1 - w1p
    # wpack[p, pass, k, j] = w1x[pass][k*128+p, j]
    wpack = (np.stack([w1b, w1p], 0)
             .reshape(2, NK, 128, J)
             .transpose(2, 0, 1, 3)
             .reshape(128, 2 * NK * J)
             .astype(ml_dtypes.bfloat16))
    # Selection matrices: route pre-tanh h rows 32f..32f+31 into a 33-row
    # block (33rd col zero; the tanh bias writes the ones row).
    selp = np.zeros((128, 4 * 33), np.float32)
    for f in range(4):
        for m in range(32):
            selp[32 * f + m, 33 * f + m] = 1.0
    sel4 = np.zeros((32, 33), np.float32)
    sel4[:32, :32] = np.eye(32)
    maas = np.stack([maa_k, maa_w, maa_v, maa_r, maa_g]).astype(np.float32)
    biasv = np.zeros((128, 1), np.float32)
    biasv[[32, 96]] = 20.0  # tanh(20) == 1.0 -> ones row for K=33 matmuls

    w2all = np.zeros((33, 5 * D), np.float32)
    for f in range(5):
        w2all[0:32, f * D:(f + 1) * D] = w2[f]
        w2all[32, f * D:(f + 1) * D] = maas[f]

    consts = {
        "wpack": wpack,
        "w2all": w2all.astype(ml_dtypes.bfloat16),

        "biasv": biasv,
        "ident": np.eye(128, dtype=np.float32),
        "selp": selp,
        "sel4": sel4,
    }
    return consts


def build_nc(T, tps=4):
    # Bacc (not bare Bass): its compile() runs move_matmul_waits_to_ldweights
    # and generate_event_semaphores, which split multi-semaphore waits to
    # satisfy the 1-wait-per-instruction TRN2 constraint walrus enforces.
    nc = bacc.Bacc("TRN2", target_bir_lowering=False, debug=False)
    nt = T // 128
    io = {
        "x": nc.dram_tensor("x", [1 + T, D], F32,
                    kind="ExternalInput").ap(),
        "xbf": nc.dram_tensor("xbf", [T, D], BF16, kind="ExternalInput").ap(),

        "haloT": nc.dram_tensor("haloT", [128, NK], BF16,
                                kind="ExternalInput").ap(),
        "wpack": nc.dram_tensor("wpack", [128, 2 * NK * J], BF16,
                                kind="ExternalInput").ap(),
        "selp": nc.dram_tensor("selp", [128, 4 * 33], F32,
                               kind="ExternalInput").ap(),
        "sel4": nc.dram_tensor("sel4", [32, 33], F32,
                               kind="ExternalInput").ap(),
        "w2all": nc.dram_tensor("w2all", [33, 5 * D], BF16,
                                kind="ExternalInput").ap(),

        "biasv": nc.dram_tensor("biasv", [128, 1], F32,
                                kind="ExternalInput").ap(),
        "ident": nc.dram_tensor("ident", [128, 128], F32,
                                kind="ExternalInput").ap(),
        "out": nc.dram_tensor("out", [T, 5, D], F32,
                              kind="ExternalOutput").ap(),
    }
    with tile.TileContext(nc) as tc:
        build_tile_program(tc, io, T, tps)
    nc.compile()
    return nc


_NC_CACHE = {}


def kernel(x, state, time_maa_x, time_maa_w1, time_maa_w2,
           maa_k, maa_w, maa_v, maa_r, maa_g, i):
    global LAST_RESULT
    x = np.asarray(x, np.float32)
    state = np.asarray(state, np.float32)
    B, S, D_ = x.shape
    assert (B, S, D_) == (4, 2048, D)
    T = S * B // N_CORES  # 1024 tokens per core
    i1 = (2 + HEAD) * int(i) + 1

    consts = host_pack_weights(time_maa_x, time_maa_w1, time_maa_w2,
                               maa_k, maa_w, maa_v, maa_r, maa_g)
    in_maps = []
    for c in range(N_CORES):
        b, half = c // 2, c % 2
        xs = np.ascontiguousarray(x[b, half * T:(half + 1) * T])
        prev = state[b, i1] if half == 0 else x[b, T - 1]
        in_maps.append({
            "x": np.concatenate([prev[None], xs], 0),
            "xbf": xs.astype(ml_dtypes.bfloat16),
            "haloT": np.ascontiguousarray(
                prev.reshape(NK, 128).T).astype(ml_dtypes.bfloat16),
            **consts})

    key = T
    if key not in _NC_CACHE:
        _NC_CACHE[key] = build_nc(T)
    nc = _NC_CACHE[key]

    global _LAST_IN_MAPS
    _LAST_IN_MAPS = in_maps
    LAST_RESULT = run_bass_kernel_spmd(nc, in_maps, list(range(N_CORES)))
    res = LAST_RESULT.results

    out = np.empty((B, S, 5, D), np.float32)
    for c in range(N_CORES):
        b, half = c // 2, c % 2
        out[b, half * T:(half + 1) * T] = res[c]["out"].reshape(T, 5, D)

    new_state = state.copy()
    new_state[:, i1] = x[:, -1]
    return out, new_state
